# revision 13
# baseline (speedup 1.0000x reference)
"""Trainium2 Bass kernel for nn_PlasticEdges (gnn_message_passing).

Math (validated vs reference):
  xufld[u, s=(y,x), c, k=(i,j)] = x[u, c, y+i-1, x+j-1]   (zero padded)
  mapped[v, s, o, k] = sum_{u,c} xufld[u,s,c,k] * mask[u,v] * chan_map[u,v,c,o]
                                 * weight[u,v,s,c,o,k]
  flat channel j = 4*k + o; fold reinterprets j as (cf, fi, fj) = (j//9,
  (j%9)//3, j%3); out[v, cf, y+fi-1, x+fj-1] += mapped[v, (y,x), o, k].

Per-core strategy (v-pair sharded across 8 cores, 2 destination nodes each):
  partitions p = (vb, u, c) = vb*64 + u*4 + c  (128 partitions)
  - stream host-pretransposed weight W[o, p, k, y, x] (18.9 MB/core)
  - DVE: tmp = W_plane * xb_plane  (elementwise)
  - TensorE: contract over p with stationary stat[p, col=vb] =
    mask*chan_map block-diagonal => psum[vb, Y, X], accumulating all 9
    shifted fold contributions per (cf, y-half) bank in PSUM.
"""

import numpy as np

_CACHE = {}


def _divmod3(j):
    return divmod(j, 3)


def _build_program():
    import concourse.bass as bass
    import concourse.tile as tile
    from concourse import bacc, mybir

    f32 = mybir.dt.float32
    nc = bacc.Bacc(None, target_bir_lowering=False)

    w_dev = nc.dram_tensor("w_dev", [4, 128, 3, 3072], f32, kind="ExternalInput")
    x_dev = nc.dram_tensor("x_dev", [128, 32, 32], f32, kind="ExternalInput")
    stat_dev = nc.dram_tensor("stat_dev", [128, 4, 2], f32, kind="ExternalInput")
    out_dev = nc.dram_tensor("out", [2, 4, 2, 16, 32], f32, kind="ExternalOutput")

    # stream order: o outer, k inner; bank (cf, h) accumulation bookkeeping
    seq = []
    for o in range(4):
        for k in range(9):
            jj = 4 * k + o
            cf = jj // 9
            for h in range(2):
                seq.append((o, k, cf, h))
    last = {}
    for idx, (o, k, cf, h) in enumerate(seq):
        last[(cf, h)] = idx

    with tile.TileContext(nc) as tc:
        with (
            tc.tile_pool(name="singles", bufs=1) as singles,
            tc.tile_pool(name="wpool", bufs=6) as wpool,
            tc.tile_pool(name="tmppool", bufs=3) as tmppool,
            tc.tile_pool(name="psump", bufs=1, space="PSUM") as psump,
        ):
            x_sb = singles.tile([128, 32, 32], f32)
            nc.sync.dma_start(out=x_sb[:], in_=x_dev[:])

            stat_sb = singles.tile([128, 4, 2], f32)
            nc.sync.dma_start(out=stat_sb[:], in_=stat_dev[:])
            zeros_sb = singles.tile([128, 512], f32)
            nc.vector.memset(zeros_sb[:], 0.0)

            # xb[kg][p, kk, y, x] = xufld plane for k = kg*3 + kk.
            # Built on the Scalar engine (otherwise idle during the stream
            # loop). A tiny "absorber" copy makes ACT observe the x DMA sem
            # first, so later ACT ops carry at most one coalesced self-wait
            # (the ISA structs have very few sync-wait slots).
            scratch = singles.tile([128, 1], f32)
            nc.scalar.copy(out=scratch[:], in_=x_sb[:, 0, 0:1])
            xb = []
            for kg in range(3):
                t = singles.tile([128, 3, 32, 32], f32, tag=f"xb{kg}", name=f"xb{kg}")
                nc.scalar.memzero(t[:].rearrange("p a y x -> p (a y x)"))
                xb.append(t)
                for kk in range(3):
                    k = kg * 3 + kk
                    i, j = divmod(k, 3)
                    ys, ye = max(0, 1 - i), min(32, 33 - i)
                    xs, xe = max(0, 1 - j), min(32, 33 - j)
                    nc.scalar.copy(
                        out=t[:, kk, ys:ye, xs:xe],
                        in_=x_sb[:, ys + i - 1 : ye + i - 1, xs + j - 1 : xe + j - 1],
                    )

            # PSUM banks: one per (cf, h); zero-init via null matmul so every
            # element has has_written set, then accumulate freely.
            ps = {}
            for cf in range(4):
                for h in range(2):
                    ps[(cf, h)] = psump.tile(
                        [2, 16, 32], f32, name=f"ps{cf}{h}", tag=f"ps{cf}{h}"
                    )
                    nc.tensor.matmul(
                        ps[(cf, h)][:].rearrange("p y x -> p (y x)"),
                        zeros_sb[:, 0:2],
                        zeros_sb[:],
                        start=True,
                        stop=False,
                    )

            # DVE absorber scratch: each absorber writes its own column so
            # absorbers never dep on each other; they exist to fold DMA/ACT
            # sem ticks into DVE's observed clock (1 wait per instruction max).
            dve_scr = singles.tile([128, 16], f32)
            scr_i = 0
            xb_seen = set()

            idx = 0
            for o in range(4):
                for kg in range(3):
                    w_kg = wpool.tile([128, 3072], f32, tag="w", name="w_kg")
                    nc.sync.dma_start(out=w_kg[:], in_=w_dev[o, :, kg])
                    nc.vector.tensor_copy(
                        out=dve_scr[:, scr_i : scr_i + 1], in_=w_kg[:, 0:1]
                    )
                    scr_i += 1
                    if kg not in xb_seen:
                        xb_seen.add(kg)
                        nc.vector.tensor_copy(
                            out=dve_scr[:, scr_i : scr_i + 1],
                            in_=xb[kg][:, 2, 16, 16:17],
                        )
                        scr_i += 1
                    tmp = tmppool.tile([128, 3072], f32, tag="tmp", name="tmp")
                    nc.vector.tensor_mul(
                        tmp[:],
                        w_kg[:],
                        xb[kg][:].rearrange("p a y x -> p (a y x)"),
                    )
                    tmp4 = tmp[:].rearrange("p (a y x) -> p a y x", a=3, y=32)
                    for kk in range(3):
                        k = kg * 3 + kk
                        jj = 4 * k + o
                        cf, r = divmod(jj, 9)
                        fi, fj = divmod(r, 3)
                        dy, dx = fi - 1, fj - 1
                        Xs, Xe = max(0, dx), min(32, 32 + dx)
                        for h in range(2):
                            Ys, Ye = max(16 * h, dy), min(16 * h + 16, 32 + dy)
                            key = (cf, h)
                            nc.tensor.matmul(
                                ps[key][:, Ys - 16 * h : Ye - 16 * h, Xs:Xe],
                                stat_sb[:, o, :],
                                tmp4[:, kk, Ys - dy : Ye - dy, Xs - dx : Xe - dx],
                                start=False,
                                stop=(last[key] == idx),
                            )
                            idx += 1

            # PSUM -> SBUF staging -> DRAM
            stage = singles.tile([2, 4, 2, 16, 32], f32)
            for cf in range(4):
                for h in range(2):
                    nc.scalar.copy(out=stage[:, cf, h, :, :], in_=ps[(cf, h)][:])
            nc.sync.dma_start(out=out_dev[:], in_=stage[:])

    nc.compile()
    return nc


def _host_prep(x, weight, chan_map, mask):
    x = np.ascontiguousarray(x, dtype=np.float32)
    weight = np.ascontiguousarray(weight, dtype=np.float32)
    chan_map = np.ascontiguousarray(chan_map, dtype=np.float32)
    mask = np.ascontiguousarray(mask, dtype=np.float32)

    # weight (u, v, s, c, o, k) -> per core [o, p=(vb,u,c), k(=kg*3+kk), y, x]
    A = weight.reshape(16, 16, 32, 32, 4, 4, 9)  # u v y x c o k
    A = A.transpose(1, 5, 0, 4, 6, 2, 3)  # v o u c k y x
    A = A.reshape(8, 2, 4, 16, 4, 9, 32, 32)  # V vb o u c k y x
    A = np.ascontiguousarray(A.transpose(0, 2, 1, 3, 4, 5, 6, 7))  # V o vb u c k y x
    w_cores = [A[V].reshape(4, 128, 3, 3072) for V in range(8)]

    cm2 = mask[:, :, None, None] * chan_map  # u v c o
    stat = np.zeros((8, 2, 16, 4, 4, 2), np.float32)  # V vb u c o col
    for vb in range(2):
        stat[:, vb, :, :, :, vb] = cm2[:, vb::2].transpose(1, 0, 2, 3)
    stat_cores = [np.ascontiguousarray(stat[V].reshape(128, 4, 2)) for V in range(8)]

    x64 = x.reshape(64, 32, 32)
    x_dev = np.ascontiguousarray(np.concatenate([x64, x64], axis=0))
    return w_cores, x_dev, stat_cores


def kernel(x, weight, chan_map, mask):
    from concourse.bass_utils import run_bass_kernel_spmd

    if "nc" not in _CACHE:
        _CACHE["nc"] = _build_program()
    nc = _CACHE["nc"]

    w_cores, x_dev, stat_cores = _host_prep(x, weight, chan_map, mask)
    in_maps = [
        {"w_dev": w_cores[V], "x_dev": x_dev, "stat_dev": stat_cores[V]}
        for V in range(8)
    ]
    res = run_bass_kernel_spmd(nc, in_maps, list(range(8)))
    outs = []
    for V in range(8):
        r = np.asarray(res.results[V]["out"])  # [2, 4, 2, 16, 32]
        outs.append(r.reshape(2, 4, 32, 32))
    return np.concatenate(outs, axis=0).astype(np.float32)


# revision 14
# speedup vs baseline: 1.7239x; 1.7239x over previous
"""Trainium2 Bass kernel for nn_PlasticEdges (gnn_message_passing).

Math (validated vs reference):
  xufld[u, s=(y,x), c, k=(i,j)] = x[u, c, y+i-1, x+j-1]   (zero padded)
  mapped[v, s, o, k] = sum_{u,c} xufld[u,s,c,k] * mask[u,v] * chan_map[u,v,c,o]
                                 * weight[u,v,s,c,o,k]
  flat channel j = 4*k + o; fold reinterprets j as (cf, fi, fj) = (j//9,
  (j%9)//3, j%3); out[v, cf, y+fi-1, x+fj-1] += mapped[v, (y,x), o, k].

Per-core strategy (v-pair sharded across 8 cores, 2 destination nodes each):
  partitions p = (vb, u, c) = vb*64 + u*4 + c  (128 partitions)
  - stream host-pretransposed weight W[o, p, k, y, x] (18.9 MB/core fp32,
    9.4 MB bf16)
  - DVE: tmp = W_plane * xb_plane  (elementwise)
  - TensorE (bf16): contract over p with stationary stat[p, col=vb] =
    mask*chan_map block-diagonal => psum[vb, Y, X], accumulating all 9
    shifted fold contributions per (cf, y-half) bank in PSUM.
"""

import numpy as np

_CACHE = {}

# "bf16w": weights/x shipped+multiplied in bf16 (halves DMA, 2x DVE mode).
# "f32w": weights stay fp32; only the matmul operands (tmp, stat) are bf16.
MODE = "bf16w"


def _build_program(mode):
    import concourse.bass as bass
    import concourse.tile as tile
    from concourse import bacc, mybir

    f32 = mybir.dt.float32
    bf16 = mybir.dt.bfloat16
    wdt = bf16 if mode == "bf16w" else f32

    nc = bacc.Bacc(None, target_bir_lowering=False)

    w_dev = nc.dram_tensor("w_dev", [4, 128, 3, 3072], wdt, kind="ExternalInput")
    x_dev = nc.dram_tensor("x_dev", [128, 32, 32], wdt, kind="ExternalInput")
    stat_dev = nc.dram_tensor("stat_dev", [128, 4, 2], bf16, kind="ExternalInput")
    out_dev = nc.dram_tensor("out", [2, 4, 2, 16, 32], f32, kind="ExternalOutput")

    # stream order: o outer, k inner; bank (cf, h) accumulation bookkeeping
    seq = []
    for o in range(4):
        for k in range(9):
            jj = 4 * k + o
            cf = jj // 9
            for h in range(2):
                seq.append((o, k, cf, h))
    last = {}
    for idx, (o, k, cf, h) in enumerate(seq):
        last[(cf, h)] = idx

    with tile.TileContext(nc) as tc:
        with (
            tc.tile_pool(name="singles", bufs=1) as singles,
            tc.tile_pool(name="wpool", bufs=6) as wpool,
            tc.tile_pool(name="tmppool", bufs=3) as tmppool,
            tc.tile_pool(name="psump", bufs=1, space="PSUM") as psump,
        ):
            x_sb = singles.tile([128, 32, 32], wdt)
            nc.sync.dma_start(out=x_sb[:], in_=x_dev[:])

            stat_sb = singles.tile([128, 4, 2], bf16)
            nc.sync.dma_start(out=stat_sb[:], in_=stat_dev[:])
            zeros_sb = singles.tile([128, 2], bf16)
            nc.vector.memset(zeros_sb[:], 0.0)

            # xb[kg][p, kk, y, x] = xufld plane for k = kg*3 + kk.
            # Built on the Scalar engine (otherwise idle during the stream
            # loop). A tiny "absorber" copy makes ACT observe the x DMA sem
            # first, so later ACT ops carry at most one coalesced self-wait
            # (the ISA structs have a single sync-wait slot).
            scratch = singles.tile([128, 1], wdt)
            nc.scalar.copy(out=scratch[:], in_=x_sb[:, 0, 0:1])
            xb = []
            for kg in range(3):
                t = singles.tile([128, 3, 32, 32], wdt, tag=f"xb{kg}", name=f"xb{kg}")
                nc.scalar.memzero(t[:].rearrange("p a y x -> p (a y x)"))
                xb.append(t)
                for kk in range(3):
                    k = kg * 3 + kk
                    i, j = divmod(k, 3)
                    ys, ye = max(0, 1 - i), min(32, 33 - i)
                    xs, xe = max(0, 1 - j), min(32, 33 - j)
                    nc.scalar.copy(
                        out=t[:, kk, ys:ye, xs:xe],
                        in_=x_sb[:, ys + i - 1 : ye + i - 1, xs + j - 1 : xe + j - 1],
                    )

            # PSUM banks: one per (cf, h). An n=1 matmul with a zero
            # stationary clears the whole bank's has_written bits, so the
            # real (start=False) matmuls overwrite-where-unset / accumulate.
            ps = {}
            for cf in range(4):
                for h in range(2):
                    ps[(cf, h)] = psump.tile(
                        [2, 16, 32], f32, name=f"ps{cf}{h}", tag=f"ps{cf}{h}"
                    )
                    nc.tensor.matmul(
                        ps[(cf, h)][:, 0:1, 0:1],
                        zeros_sb[:],
                        zeros_sb[:, 0:1],
                        start=True,
                        stop=False,
                    )

            # DVE absorber scratch: each absorber writes its own column so
            # absorbers never dep on each other; they fold DMA/ACT sem ticks
            # into DVE's observed clock (1 wait per instruction max).
            dve_scr = singles.tile([128, 16], f32)
            scr_i = 0
            xb_seen = set()

            idx = 0
            for o in range(4):
                for kg in range(3):
                    w_kg = wpool.tile([128, 3072], wdt, tag="w", name="w_kg")
                    nc.sync.dma_start(out=w_kg[:], in_=w_dev[o, :, kg])
                    nc.vector.tensor_copy(
                        out=dve_scr[:, scr_i : scr_i + 1], in_=w_kg[:, 0:1]
                    )
                    scr_i += 1
                    if kg not in xb_seen:
                        xb_seen.add(kg)
                        nc.vector.tensor_copy(
                            out=dve_scr[:, scr_i : scr_i + 1],
                            in_=xb[kg][:, 2, 16, 16:17],
                        )
                        scr_i += 1
                    tmp = tmppool.tile([128, 3072], bf16, tag="tmp", name="tmp")
                    nc.vector.tensor_mul(
                        tmp[:],
                        w_kg[:],
                        xb[kg][:].rearrange("p a y x -> p (a y x)"),
                    )
                    tmp4 = tmp[:].rearrange("p (a y x) -> p a y x", a=3, y=32)
                    for kk in range(3):
                        k = kg * 3 + kk
                        jj = 4 * k + o
                        cf, r = divmod(jj, 9)
                        fi, fj = divmod(r, 3)
                        dy, dx = fi - 1, fj - 1
                        Xs, Xe = max(0, dx), min(32, 32 + dx)
                        for h in range(2):
                            Ys, Ye = max(16 * h, dy), min(16 * h + 16, 32 + dy)
                            key = (cf, h)
                            nc.tensor.matmul(
                                ps[key][:, Ys - 16 * h : Ye - 16 * h, Xs:Xe],
                                stat_sb[:, o, :],
                                tmp4[:, kk, Ys - dy : Ye - dy, Xs - dx : Xe - dx],
                                start=False,
                                stop=(last[key] == idx),
                            )
                            idx += 1

            # PSUM -> SBUF staging -> DRAM (all on ACT: single-wait coalescing)
            stage = singles.tile([2, 4, 2, 16, 32], f32)
            for cf in range(4):
                for h in range(2):
                    nc.scalar.copy(out=stage[:, cf, h, :, :], in_=ps[(cf, h)][:])
            nc.sync.dma_start(out=out_dev[:], in_=stage[:])

    nc.compile()
    return nc


def _host_prep(x, weight, chan_map, mask, mode):
    import ml_dtypes

    wnp = ml_dtypes.bfloat16 if mode == "bf16w" else np.float32

    x = np.ascontiguousarray(x, dtype=np.float32)
    weight = np.ascontiguousarray(weight, dtype=np.float32)
    chan_map = np.ascontiguousarray(chan_map, dtype=np.float32)
    mask = np.ascontiguousarray(mask, dtype=np.float32)

    # weight (u, v, s, c, o, k) -> per core [o, p=(vb,u,c), k(=kg*3+kk), y, x]
    A = weight.reshape(16, 16, 32, 32, 4, 4, 9)  # u v y x c o k
    A = A.transpose(1, 5, 0, 4, 6, 2, 3)  # v o u c k y x
    A = A.reshape(8, 2, 4, 16, 4, 9, 32, 32)  # V vb o u c k y x
    A = np.ascontiguousarray(A.transpose(0, 2, 1, 3, 4, 5, 6, 7), dtype=wnp)
    w_cores = [A[V].reshape(4, 128, 3, 3072) for V in range(8)]

    cm2 = mask[:, :, None, None] * chan_map  # u v c o
    stat = np.zeros((8, 2, 16, 4, 4, 2), np.float32)  # V vb u c o col
    for vb in range(2):
        stat[:, vb, :, :, :, vb] = cm2[:, vb::2].transpose(1, 0, 2, 3)
    stat = stat.astype(ml_dtypes.bfloat16)
    stat_cores = [np.ascontiguousarray(stat[V].reshape(128, 4, 2)) for V in range(8)]

    x64 = x.reshape(64, 32, 32)
    x_dev = np.ascontiguousarray(np.concatenate([x64, x64], axis=0), dtype=wnp)
    return w_cores, x_dev, stat_cores


def kernel(x, weight, chan_map, mask):
    from concourse.bass_utils import run_bass_kernel_spmd

    if "nc" not in _CACHE:
        _CACHE["nc"] = _build_program(MODE)
    nc = _CACHE["nc"]

    w_cores, x_dev, stat_cores = _host_prep(x, weight, chan_map, mask, MODE)
    in_maps = [
        {"w_dev": w_cores[V], "x_dev": x_dev, "stat_dev": stat_cores[V]}
        for V in range(8)
    ]
    res = run_bass_kernel_spmd(nc, in_maps, list(range(8)))
    outs = []
    for V in range(8):
        r = np.asarray(res.results[V]["out"])  # [2, 4, 2, 16, 32]
        outs.append(r.reshape(2, 4, 32, 32))
    return np.concatenate(outs, axis=0).astype(np.float32)
